# revision 2
# baseline (speedup 1.0000x reference)
"""Trainium2 Bass kernel for the rank-weighted log-loss reduction.

loss = -sum_i ri * (log(p_win_i) - R*(f0_i - P1)^2),  ri = i / (n*(n+1)/2)

Strategy (pure data parallel over 8 cores):
  - core k gets rows [k*M, (k+1)*M), M = N/8.
  - per tile (128 partitions x F rows): ACT computes ln0=Ln(f0), ln1=Ln(f1),
    sq=(f0-P1)^2 (all bf16, X-dependent only); DVE does one predicated copy
    ln0 <- ln1 where pv!=0 (the only V-dependent op); PE accumulates
    Wp^T*sq + Wn^T*ln into PSUM acc[4, 1024] (q = sq - ln = -per).
  - weight columns (1, b0, b1, b2) encode the row base v = B_t + F_t*p
    exactly in bf16 via a 3-byte split (v < 2^21).
  - ramp-down tile sizes keep the post-last-DMA serial chain tiny; PSUM
    column ranges are drained to SBUF as their accumulation finishes and
    DMA'd out from the SP queue.
  - host folds [4, 1024] per-core partials in float64.
"""

import numpy as np
import ml_dtypes
from contextlib import ExitStack

import concourse.bass as bass
import concourse.mybir as mybir
import concourse.tile as tile
from concourse.bass_utils import run_bass_kernel_spmd


MAX_SYNC_WAITS = 1


def _spill_excess_waits(nc, max_waits=None):
    max_waits = max_waits or MAX_SYNC_WAITS
    """The walrus in this toolchain rejects instructions carrying more than
    a couple of sync waits ("Too many sync wait commands"). Spill the excess
    onto same-engine NOPs inserted immediately before — semantically
    identical (consecutive sem-ge waits on one engine)."""
    import bass_rust

    k = 0
    for f in nc.m.functions:
        for b in f.blocks:
            out = []
            changed = False
            for inst in b.instructions:
                si = inst.sync_info
                waits = list(si.on_wait or []) if si is not None else []
                if len(waits) > max_waits:
                    chunks = [
                        waits[i : i + max_waits]
                        for i in range(0, len(waits), max_waits)
                    ]
                    for chunk in chunks[:-1]:
                        nop = mybir.InstNoOp(name=f"antspill-{k}", ins=[], outs=[])
                        k += 1
                        nop.engine = inst.engine
                        nop.sync_info = bass_rust.SyncInfo(
                            on_wait=chunk, on_update=[]
                        )
                        out.append(nop)
                    inst.sync_info = bass_rust.SyncInfo(
                        on_wait=chunks[-1], on_update=list(si.on_update or [])
                    )
                    changed = True
                out.append(inst)
            if changed:
                b.instructions = out


N_TOTAL = 16777216
N_CORES = 8
P = 128
FW = 1024                      # accumulator / output width
F_LIST = [1024] * 15 + [512, 512]   # sum = 16384 = M / P
R = 1.0
P1 = 0.5

# column ranges of acc and the tile index after which each is final
def _drain_plan(f_list, extra_bounds=()):
    """[(c0, c1, last_tile_idx)] — range [c0,c1) is complete after tile t's
    matmuls iff no later tile has F > c0. extra_bounds adds split points so
    the final tile's drain can be pipelined against its own matmuls."""
    bounds = sorted({0, *f_list, FW, *extra_bounds})
    plan = []
    for c0, c1 in zip(bounds[:-1], bounds[1:]):
        last = max(t for t, F in enumerate(f_list) if F > c0)
        plan.append((c0, c1, last))
    return plan


def _mm_chunks(F, plan, t):
    """Column chunks for tile t's matmuls: <=512 wide (PSUM bank), split at
    boundaries of ranges whose accumulation stops at this tile so the stop
    flag lands on the exact range."""
    cuts = {c0 for c0, _, last in plan if last == t and c0 > 0}
    if F > 512:
        cuts.add(512)
    cuts.add(F)
    chunks = []
    c0 = 0
    for c1 in sorted(cuts):
        if c1 > c0:
            chunks.append((c0, c1))
            c0 = c1
    return chunks


def _patch_prep_dma_waits(nc, sem_name="swdge_dma", count=32):
    """PREPARE_ONLY scatters carry a user completion sem in the descriptor's
    single SDMA-sem slot, so the DMASW lane ticks the tile framework assigned
    them never fire — but the exit barrier still waits on those lanes.
    Rewrite any wait on a never-updated DMASW sem to wait on the user sem
    reaching `count` (16 per scatter), which is the true completion signal."""
    import bass_rust

    updated, sem_id = set(), None
    for f in nc.m.functions:
        for b in f.blocks:
            for inst in b.instructions:
                si = inst.sync_info
                if si is None:
                    continue
                for u in si.on_update or []:
                    updated.add(u.ant_name)
                    if u.ant_name == sem_name:
                        sem_id = u.id
    assert sem_id is not None
    for f in nc.m.functions:
        for b in f.blocks:
            for inst in b.instructions:
                si = inst.sync_info
                if si is None or not si.on_wait:
                    continue
                ws, changed = [], False
                for w in si.on_wait:
                    if (w.ant_name or "").startswith("DMASW") and \
                            w.ant_name not in updated:
                        ws.append(bass_rust.SyncWait(
                            sync_type="semaphore", id=sem_id, ant_name=sem_name,
                            wait_mode="sem-ge-imm", wait_value=count,
                        ))
                        changed = True
                    else:
                        ws.append(w)
                if changed:
                    inst.sync_info = bass_rust.SyncInfo(
                        on_wait=ws, on_update=list(si.on_update or [])
                    )


def build_nc(f_list=F_LIST, extra_bounds=(), rev_chunks=False, use_scatter=False):
    T = len(f_list)
    M = P * sum(f_list)
    nc = bass.Bass(
        "TRN2", target_bir_lowering=False, debug=False,
        enable_asserts=False, num_devices=1,
    )
    fo = nc.dram_tensor("fo", [M, 2], mybir.dt.float32, kind="ExternalInput")
    pv = nc.dram_tensor("pv", [M], mybir.dt.int32, kind="ExternalInput")
    wt = nc.dram_tensor("wt", [P, 8 * T], mybir.dt.bfloat16, kind="ExternalInput")

    plan = _drain_plan(f_list, extra_bounds)
    stop_at = {(last, (c0, c1)) for c0, c1, last in plan}
    # output split: the last-to-finalize (lowest) range goes to its own
    # scatter so the final drain->out hop skips the HWDGE config pipeline
    lo_w = min(c1 for c0, c1, last in plan if c0 == 0)
    hi_w = FW - lo_w
    if use_scatter:
        out_lo = nc.dram_tensor("out_lo", [4, lo_w], mybir.dt.float32,
                                kind="ExternalOutput")
        out_hi = nc.dram_tensor("out_hi", [4, hi_w], mybir.dt.float32,
                                kind="ExternalOutput")
    else:
        out = nc.dram_tensor("out", [4, FW], mybir.dt.float32,
                             kind="ExternalOutput")

    with tile.TileContext(nc) as tc, ExitStack() as ctx:
        xp = ctx.enter_context(tc.tile_pool(name="xp", bufs=4))
        vp = ctx.enter_context(tc.tile_pool(name="vp", bufs=4))
        mp = ctx.enter_context(tc.tile_pool(name="mp", bufs=4))
        cp = ctx.enter_context(tc.tile_pool(name="cp", bufs=1))
        tp = ctx.enter_context(tc.tile_pool(name="tp", bufs=1))
        ps = ctx.enter_context(tc.tile_pool(name="ps", bufs=1, space="PSUM"))

        # stationary weights + scatter indices via Pool SWDGE — keeps SP's
        # queue free for the input stream so the first X descgen starts
        # immediately
        W = cp.tile([P, 8 * T], mybir.dt.bfloat16)
        nc.gpsimd.dma_start(W[:], wt[:])
        if use_scatter:
            ix = nc.dram_tensor("ix", [16, 8], mybir.dt.int16,
                                kind="ExternalInput")
            IX = cp.tile([16, 8], mybir.dt.int16)
            nc.gpsimd.dma_start(IX[:], ix[:])
        acc = ps.tile([4, FW], mybir.dt.float32)

        # scatter-add staging: drains land here; the output DMAs are
        # PREPARE_ONLY scatters whose descriptors are generated up front, so
        # the final drain->HBM hop is just a cheap trigger + transfer
        # (skipping the 650ns seq config + 625ns descgen + 650ns DGE delay)
        if use_scatter:
            sb_lo = cp.tile([P, 1, lo_w], mybir.dt.float32)
            sb_hi = cp.tile([P, 1, hi_w], mybir.dt.float32)
            dsem = nc.alloc_semaphore("swdge_dma")
            nc.gpsimd.dma_scatter_add(
                out_hi[:], sb_hi[:], IX[:, 0:1], 4, 4, hi_w,
                prepare_only=True, sem=dsem,
            )
            nc.gpsimd.dma_scatter_add(
                out_lo[:], sb_lo[:], IX[:, 0:1], 4, 4, lo_w,
                prepare_only=True, sem=dsem,
            )
        else:
            outsb = cp.tile([4, FW], mybir.dt.float32)

        bases = np.cumsum([0] + [P * F for F in f_list]).tolist()

        def fo_ap(t):
            return fo.ap()[bases[t] : bases[t + 1]].rearrange(
                "(p f) c -> p f c", p=P
            )

        def pv_ap(t):
            return pv.ap()[bases[t] : bases[t + 1]].rearrange("(p f) -> p f", p=P)

        def x_side(t, F, X):
            """X-dependent per-tile compute: lns on ACT; sq = (f0-P1)^2 split
            across Pool (subtract; tensor_scalar is ~1 elem/cycle there) and
            DVE (self-multiply), keeping ACT at two ops per tile."""
            Xs = X[:, :F, :]
            ln0, ln1 = tiles[t]["ln0"][:, :F], tiles[t]["ln1"][:, :F]
            dm, sq = tiles[t]["dm"][:, :F], tiles[t]["sq"][:, :F]
            nc.scalar.activation(ln0, Xs[:, :, 0], mybir.ActivationFunctionType.Ln)
            nc.scalar.activation(ln1, Xs[:, :, 1], mybir.ActivationFunctionType.Ln)
            nc.gpsimd.tensor_scalar_add(dm, Xs[:, :, 0], -P1)
            nc.vector.tensor_tensor(sq, dm, dm, mybir.AluOpType.mult)

        def pred(t, F):
            ln0, ln1 = tiles[t]["ln0"][:, :F], tiles[t]["ln1"][:, :F]
            nc.vector.copy_predicated(ln0, tiles[t]["V"][:, :F], ln1)

        def sq_mms(t, F):
            sq = tiles[t]["sq"][:, :F]
            Wp = W[:, 8 * t : 8 * t + 4]
            # chunks descending so the last tile's low range stops last and
            # its drain is the only thing between the final matmul and the
            # output trigger
            chunks = _mm_chunks(F, plan, t)
            for c0, c1 in (reversed(chunks) if rev_chunks else chunks):
                # t == 0 covers every acc address (F_0 == FW), so each
                # address gets start=True exactly once
                nc.tensor.matmul(
                    acc[:, c0:c1], Wp, sq[:, c0:c1], start=(t == 0), stop=False,
                    skip_group_check=True,
                )

        def ln_mms(t, F):
            ln0 = tiles[t]["ln0"][:, :F]
            Wn = W[:, 8 * t + 4 : 8 * t + 8]
            chunks = _mm_chunks(F, plan, t)
            for c0, c1 in (reversed(chunks) if rev_chunks else chunks):
                nc.tensor.matmul(
                    acc[:, c0:c1], Wn, ln0[:, c0:c1], start=False,
                    stop=(t, (c0, c1)) in stop_at, skip_group_check=True,
                )

        def v_side(t, F):
            pred(t, F)
            sq_mms(t, F)
            ln_mms(t, F)

        def drain(t):
            """PSUM ranges finalized by tile t, copied on ACT (idle by the
            time the tail runs) so the DVE pred ladder is never stalled.
            Descending, matching the order the stops complete."""
            ranges = [r for r in plan if r[2] == t]
            for c0, c1, last in (reversed(ranges) if rev_chunks else ranges):
                if True:
                    if use_scatter:
                        if c0 >= lo_w:
                            dst = sb_hi[0:4, 0, c0 - lo_w : c1 - lo_w]
                        else:
                            dst = sb_lo[0:4, 0, c0:c1]
                    else:
                        dst = outsb[:, c0:c1]
                    nc.scalar.copy(dst, acc[:, c0:c1])
                    drained.append((c0, c1))

        tiles = {}
        drained = []   # (c0, c1) ranges already copied to outsb
        n_main = sum(1 for F in f_list if F == FW)
        tail = list(range(n_main, len(f_list)))

        # main tiles stream (X_t, V_t) pairs with rotating buffers
        for t in range(n_main):
            F = f_list[t]
            X = xp.tile([P, FW, 2], mybir.dt.float32, tag="X")
            V = vp.tile([P, FW], mybir.dt.int32, tag="V")
            ln0_t = mp.tile([P, FW], mybir.dt.bfloat16, tag="ln0")
            ln1_t = mp.tile([P, FW], mybir.dt.bfloat16, tag="ln1")
            dm_t = mp.tile([P, FW], mybir.dt.float32, tag="dm")
            sq_t = mp.tile([P, FW], mybir.dt.bfloat16, tag="sq")
            tiles[t] = {"X": X, "V": V, "ln0": ln0_t, "ln1": ln1_t,
                        "dm": dm_t, "sq": sq_t}
            nc.sync.dma_start(X[:, :F, :], fo_ap(t))
            nc.sync.dma_start(V[:, :F], pv_ap(t))
            x_side(t, F, X)
            v_side(t, F)
            drain(t)

        # ramp-down tiles get dedicated buffers (no reuse waits). Their X
        # tensors stream right after the main tiles; the last bytes on the
        # wire are the tail V tensors, whose only consumer is the tiny
        # predicated select — so the post-stream serial chain is minimal.
        for t in tail:
            F = f_list[t]
            tiles[t] = {
                "X": tp.tile([P, F, 2], mybir.dt.float32, tag=f"tX{t}",
                             name=f"tX{t}"),
                "V": tp.tile([P, F], mybir.dt.int32, tag=f"tV{t}",
                             name=f"tV{t}"),
                "ln0": tp.tile([P, F], mybir.dt.bfloat16, tag=f"tl0{t}",
                               name=f"tl0{t}"),
                "ln1": tp.tile([P, F], mybir.dt.bfloat16, tag=f"tl1{t}",
                               name=f"tl1{t}"),
                "dm": tp.tile([P, F], mybir.dt.float32, tag=f"tdm{t}",
                              name=f"tdm{t}"),
                "sq": tp.tile([P, F], mybir.dt.bfloat16, tag=f"tsq{t}",
                              name=f"tsq{t}"),
            }
            nc.sync.dma_start(tiles[t]["X"][:, :F, :], fo_ap(t))
        for t in tail:
            x_side(t, f_list[t], tiles[t]["X"])
            sq_mms(t, f_list[t])
        for t in tail:
            nc.sync.dma_start(tiles[t]["V"][:, :f_list[t]], pv_ap(t))
        for t in tail:
            pred(t, f_list[t])
        for t in tail:
            ln_mms(t, f_list[t])
        for t in tail:
            drain(t)

        if use_scatter:
            # fire both prepared output scatters; the tile framework defers
            # the staging RAW edges onto this trigger, so it waits exactly
            # for the drain copies
            nc.gpsimd.trigger_dma(count=None)
        else:
            # out DMAs on SP after all input issues: the big one's config
            # overlaps the tail compute; the final one waits only its drain
            nc.sync.dma_start(out[:, lo_w:], outsb[:, lo_w:])
            nc.sync.dma_start(out[:, :lo_w], outsb[:, :lo_w])
    if use_scatter:
        _patch_prep_dma_waits(nc)
    _spill_excess_waits(nc)
    return nc


def build_wt(f_list=F_LIST):
    """Per-tile stationary matrix: columns (1, b0, b1, b2) then the negated
    four. v = B_t + F_t*p is the row index of (p, f=0) in this core's shard;
    the 3-byte split keeps every component exact in bf16 (v < 2^21)."""
    T = len(f_list)
    cols = np.zeros((P, 8 * T), np.float32)
    p_idx = np.arange(P, dtype=np.int64)
    base = 0
    for t, F in enumerate(f_list):
        v = base + F * p_idx
        assert v.max() < (1 << 21)
        cols[:, 8 * t + 0] = 1.0
        cols[:, 8 * t + 1] = v & 0xFF
        cols[:, 8 * t + 2] = v & 0xFF00
        cols[:, 8 * t + 3] = v & 0xFF0000
        cols[:, 8 * t + 4 : 8 * t + 8] = -cols[:, 8 * t : 8 * t + 4]
        base += P * F
    return cols.astype(ml_dtypes.bfloat16)


def build_ix():
    """Scatter token indices: token i sits at [i % 16, i // 16]; tokens 0-3
    map staging partitions 0-3 to output rows 0-3, the rest are ignored."""
    ix = np.full((16, 8), -1, np.int16)
    ix[0:4, 0] = np.arange(4, dtype=np.int16)
    return ix


def combine(outs, f_list=F_LIST):
    """Fold per-core [4, FW] partials into the loss.

    acc accumulated q = sq - ln = -per with weights (1, b-split of v), where
    local row i = v + f. So per core:
      sum_i i*per_i = -(sum_f f*r0[f] + sum rb),  sum_i per_i = -sum r0.
    """
    M = P * sum(f_list)
    n = M * len(outs)
    denom = float(np.float32(n) * np.float32(n + 1) * np.float32(0.5))
    j = np.arange(FW, dtype=np.float64)
    total = 0.0
    for k, o in enumerate(outs):
        r0 = o[0].astype(np.float64)
        rb = (o[1] + o[2] + o[3]).astype(np.float64)
        total += (k * M) * r0.sum() + (j * r0).sum() + rb.sum()
    return total / denom


_NC_CACHE = {}


def _run(final_out, point_victor, **spmd_kwargs):
    fo = np.ascontiguousarray(np.asarray(final_out, dtype=np.float32))
    pv = np.ascontiguousarray(np.asarray(point_victor, dtype=np.int32))
    assert fo.shape == (N_TOTAL, 2) and pv.shape == (N_TOTAL,)
    M = N_TOTAL // N_CORES

    if "nc" not in _NC_CACHE:
        _NC_CACHE["nc"] = build_nc()
        _NC_CACHE["use_scatter"] = False
    nc = _NC_CACHE["nc"]
    wt = build_wt()
    ixm = build_ix()

    in_maps = [
        {"fo": fo[k * M : (k + 1) * M], "pv": pv[k * M : (k + 1) * M],
         "wt": wt}
        for k in range(N_CORES)
    ]
    if _NC_CACHE.get("use_scatter"):
        for m in in_maps:
            m["ix"] = ixm
    res = run_bass_kernel_spmd(nc, in_maps, core_ids=list(range(N_CORES)), **spmd_kwargs)
    if _NC_CACHE.get("use_scatter"):
        outs = [
            np.concatenate([r["out_lo"], r["out_hi"]], axis=1)
            for r in res.results
        ]
    else:
        outs = [r["out"] for r in res.results]
    return np.float32(combine(outs)), res


def kernel(final_out, point_victor):
    return _run(final_out, point_victor)[0]
